# revision 65
# baseline (speedup 1.0000x reference)
"""Batched KDE kernel for Trainium2 (8 NeuronCores, SPMD).

Problem: out[b, n] = sum_m exp(-||Xq[b,n] - Xf[b,m]||^2 / bw[b])
  with Silverman bandwidth bw[b] from Xf; b=4, n=m=4096, d=32.

Sharding: data-parallel over batch b (4 batches x 2 shards of query rows
= 8 cores). Each core handles n_shard=2048 query rows against the full
m=4096 fit set of its batch.

Device algorithm (per core), raw Bass with manual semaphores:
  psum[n, m] = 2*dot - nmu2 via TWO bf16 K=128 matmuls per 512-col chunk
  (bf16 streams at 1 col/cycle; f32 values are split into bf16 pieces
  x = x1+x2+x3+O(2^-24); Q = 2*Xq^T, f = Xf^T, s = f32(f^2)):
    mmA: lhsT=[q1; q1; q1; -1]   rhs=[f1; f2; f3; s1]
    mmB: lhsT=[q2; q2; q3; -1]   rhs=[f1; f2; f1; s2]
  Sum = q1(f1+f2+f3) + q2(f1+f2) + q3*f1 - s1 - s2 = Q*f - s up to
  O(2^-17)-level dropped cross terms (~2e-4 relative on the exp args).
  ScalarE activation computes exp(psum/bw - nx2/bw) with a fused
  per-partition accumulate (accum_out) -> the sum over m. ACT is the
  bottleneck engine (~1 elem/lane/cycle @ 1.2 GHz).
  nx2 (query norms) is computed on-device from the raw query rows.
Host does sharding/layout/packing plus the 4 scalar bandwidth values
(the global quantile needs a sort, which is pathological on-device).
"""

import os
import numpy as np

B, N, M, D = 4, 4096, 4096, 32
NCORES = 8
SHARDS_PER_BATCH = NCORES // B  # 2
NSHARD = N // SHARDS_PER_BATCH  # 2048
NT = NSHARD // 128  # 16 n-tiles per core
MCHUNK = 512  # matmul free-dim chunk (one psum bank)
ACT_FD = 2048  # activation free dim (4 psum banks)
NG = NT * (M // ACT_FD)  # 32 matmul/exp groups
CPG = ACT_FD // MCHUNK  # psum banks per group = 4

_cached = {}


def _build_program():
    import concourse.bass as bass
    import concourse.mybir as mybir
    from contextlib import ExitStack

    nc = bass.Bass()
    f32 = mybir.dt.float32
    bf16 = mybir.dt.bfloat16

    # stationary operands: only the 96 data rows come from the host; the
    # -1 rows (96:128) are memset on-device
    la = nc.declare_dram_parameter("la", [96, NSHARD], bf16, isOutput=False)
    lb = nc.declare_dram_parameter("lb", [96, NSHARD], bf16, isOutput=False)
    ra = nc.declare_dram_parameter("ra", [128, M], bf16, isOutput=False)
    rb = nc.declare_dram_parameter("rb", [128, M], bf16, isOutput=False)
    XQN_W = NT * D + 1 + NT
    xqn = nc.declare_dram_parameter("xqn", [128, XQN_W], f32, isOutput=False)
    res = nc.declare_dram_parameter("res", [128, NT], f32, isOutput=True)

    NLC = 2  # la/lb chunks: 256 cols then 1792 cols
    NRC = 4  # 1024-col chunks of ra/rb
    LSPLIT = 256
    RW = M // NRC

    with ExitStack() as ctx:
        la_sb = ctx.enter_context(nc.sbuf_tensor([128, NSHARD], bf16))
        lb_sb = ctx.enter_context(nc.sbuf_tensor([128, NSHARD], bf16))
        ra_sb = ctx.enter_context(nc.sbuf_tensor([128, M], bf16))
        rb_sb = ctx.enter_context(nc.sbuf_tensor([128, M], bf16))
        xqn_sb = ctx.enter_context(nc.sbuf_tensor([128, XQN_W], f32))
        sq = ctx.enter_context(nc.sbuf_tensor([128, NT * D], f32))
        nx2r = ctx.enter_context(nc.sbuf_tensor([128, NT], f32))
        bias_all = ctx.enter_context(nc.sbuf_tensor([128, NT], f32))
        # slot 0..NG-1 = regular groups; slot NG = the split-off first
        # half-group (banks 0-1 of group 0, summed into res col 0 at the end)
        acc = ctx.enter_context(nc.sbuf_tensor([128, NG + 1], f32))
        res_sb = ctx.enter_context(nc.sbuf_tensor([128, NT], f32))
        warmT = ctx.enter_context(nc.sbuf_tensor([1, 1], f32))
        escr0 = ctx.enter_context(nc.sbuf_tensor([128, ACT_FD], bf16))
        escr1 = ctx.enter_context(nc.sbuf_tensor([128, ACT_FD], bf16))
        escr = [escr0, escr1]
        ps0 = ctx.enter_context(nc.psum_tensor("ps0", [128, ACT_FD], f32))
        ps1 = ctx.enter_context(nc.psum_tensor("ps1", [128, ACT_FD], f32))
        ps = [ps0, ps1]

        sem_xqn = ctx.enter_context(nc.semaphore("sem_xqn"))
        s_la = [ctx.enter_context(nc.semaphore(f"s_la{i}")) for i in range(NLC)]
        s_lb = [ctx.enter_context(nc.semaphore(f"s_lb{i}")) for i in range(NLC)]
        s_ra = [ctx.enter_context(nc.semaphore(f"s_ra{i}")) for i in range(NRC)]
        s_rb = [ctx.enter_context(nc.semaphore(f"s_rb{i}")) for i in range(NRC)]
        sem_out = ctx.enter_context(nc.semaphore("sem_out"))
        s_warm = ctx.enter_context(nc.semaphore("s_warm"))
        s_dve = ctx.enter_context(nc.semaphore("s_dve"))
        s_act = ctx.enter_context(nc.semaphore("s_act"))
        s_pe = ctx.enter_context(nc.semaphore("s_pe"))
        block = ctx.enter_context(nc.Block())

        scale_pos = xqn_sb[:, NT * D : NT * D + 1]  # 1/bw
        neg_invbw = xqn_sb[:, NT * D + 1 : NT * D + 1 + NT]  # -1/bw x NT

        @block.sync
        def _(sync):
            # critical first-half chunks; the rest is deferred until these
            # have landed so they don't compete for HBM bandwidth (the
            # second m-half isn't consumed until halfway through the run)
            sync.dma_start(xqn_sb[:], xqn[:]).then_inc(sem_xqn, 16)
            sync.dma_start(la_sb[0:96, 0:LSPLIT], la[:, 0:LSPLIT]).then_inc(
                s_la[0], 16
            )
            sync.dma_start(lb_sb[0:96, 0:LSPLIT], lb[:, 0:LSPLIT]).then_inc(
                s_lb[0], 16
            )
            sync.dma_start(ra_sb[:, 0:RW], ra[:, 0:RW]).then_inc(s_ra[0], 16)
            sync.dma_start(rb_sb[:, 0:RW], rb[:, 0:RW]).then_inc(s_rb[0], 16)
            sync.dma_start(
                ra_sb[:, RW : 2 * RW], ra[:, RW : 2 * RW]
            ).then_inc(s_ra[1], 16)
            sync.dma_start(
                rb_sb[:, RW : 2 * RW], rb[:, RW : 2 * RW]
            ).then_inc(s_rb[1], 16)
            sync.dma_start(
                la_sb[0:96, LSPLIT:NSHARD], la[:, LSPLIT:NSHARD]
            ).then_inc(s_la[1], 16)
            sync.dma_start(
                lb_sb[0:96, LSPLIT:NSHARD], lb[:, LSPLIT:NSHARD]
            ).then_inc(s_lb[1], 16)
            sync.wait_ge(s_rb[1], 16)
            for i in range(2, NRC):
                sync.dma_start(
                    ra_sb[:, i * RW : (i + 1) * RW], ra[:, i * RW : (i + 1) * RW]
                ).then_inc(s_ra[i], 16)
                sync.dma_start(
                    rb_sb[:, i * RW : (i + 1) * RW], rb[:, i * RW : (i + 1) * RW]
                ).then_inc(s_rb[i], 16)
            sync.wait_ge(s_dve, 8)
            sync.dma_start(res[:], res_sb[:]).then_inc(sem_out, 16)
            sync.wait_ge(sem_out, 16)

        @block.vector
        def _(vector):
            # scratch for the PE warmup matmuls
            nc.vector.memset(escr0[:, 0:MCHUNK], 0.0).then_inc(s_dve, 1)
            # -1 rows of the stationary operands
            nc.vector.memset(la_sb[96:128, :], -1.0).then_inc(s_dve, 1)
            nc.vector.memset(lb_sb[96:128, :], -1.0).then_inc(s_dve, 1)
            vector.wait_ge(sem_xqn, 16)
            nc.vector.tensor_tensor(
                sq[:],
                xqn_sb[:, : NT * D],
                xqn_sb[:, : NT * D],
                op=mybir.AluOpType.mult,
            ).then_inc(s_dve, 1)
            vector.wait_ge(s_dve, 4)
            nc.vector.tensor_reduce(
                nx2r[:],
                sq[:].rearrange("p (t d) -> p t d", d=D),
                axis=mybir.AxisListType.X,
                op=mybir.AluOpType.add,
            ).then_inc(s_dve, 1)
            vector.wait_ge(s_dve, 5)
            nc.vector.tensor_tensor(
                bias_all[:], nx2r[:], neg_invbw, op=mybir.AluOpType.mult
            ).then_inc(s_dve, 1)
            vector.wait_ge(s_act, NG + 1)
            nc.vector.tensor_reduce(
                res_sb[:],
                acc[:, :NG].rearrange("p (t h) -> p t h", h=M // ACT_FD),
                axis=mybir.AxisListType.X,
                op=mybir.AluOpType.add,
            ).then_inc(s_dve, 1)
            vector.wait_ge(s_dve, 7)
            nc.vector.tensor_tensor(
                res_sb[:, 0:1],
                res_sb[:, 0:1],
                acc[:, NG : NG + 1],
                op=mybir.AluOpType.add,
            ).then_inc(s_dve, 1)

        @block.scalar
        def _(scalar):
            # trigger the exp table-set DMA during the kernel head so the
            # first real exp doesn't pay the ~1.3us ACT_TABLE_LOAD
            nc.scalar.memzero(warmT[:]).then_inc(s_warm, 1)
            scalar.wait_ge(s_warm, 1)
            nc.scalar.activation(
                warmT[:], warmT[:], mybir.ActivationFunctionType.Exp
            )
            scalar.wait_ge(sem_xqn, 16)
            scalar.wait_ge(s_dve, 6)
            # group 0 is split in two so the first exp can start as soon as
            # the first 1024 columns of psum are ready
            scalar.wait_ge(s_pe, 1)
            nc.scalar.activation(
                escr[0][:, 0 : ACT_FD // 2],
                ps[0][:, 0 : ACT_FD // 2],
                mybir.ActivationFunctionType.Exp,
                bias=bias_all[:, 0:1],
                scale=scale_pos,
                accum_out=acc[:, NG : NG + 1],
            ).then_inc(s_act, 1)
            scalar.wait_ge(s_pe, 2)
            nc.scalar.activation(
                escr[0][:, ACT_FD // 2 :],
                ps[0][:, ACT_FD // 2 :],
                mybir.ActivationFunctionType.Exp,
                bias=bias_all[:, 0:1],
                scale=scale_pos,
                accum_out=acc[:, 0:1],
            ).then_inc(s_act, 1)
            for g in range(1, NG):
                t = g % NT
                scalar.wait_ge(s_pe, g + 2)
                slot = t * (M // ACT_FD) + (g // NT)
                nc.scalar.activation(
                    escr[g % 2][:],
                    ps[g % 2][:],
                    mybir.ActivationFunctionType.Exp,
                    bias=bias_all[:, t : t + 1],
                    scale=scale_pos,
                    accum_out=acc[:, slot : slot + 1],
                ).then_inc(s_act, 1)

        @block.tensor
        def _(tensor):
            # warm the PE clock (HAM) during the head with dummy matmuls on
            # the memset -1 rows, so the first real groups run at 2.4 GHz
            tensor.wait_ge(s_dve, 1)  # warmup scratch memset
            for _w in range(16):
                nc.tensor.matmul(
                    ps0[:, 0:MCHUNK],
                    escr0[:, 0:128],
                    escr0[:, 0:MCHUNK],
                    start=True,
                    stop=True,
                )
            tensor.wait_ge(s_dve, 3)  # la/lb -1 rows memset
            g = 0
            # h-outer: the second m-half (ra/rb chunks 2-3) is first
            # touched halfway through the kernel, so its DMA is deferred
            for h in range(M // ACT_FD):
                for t in range(NT):
                    if t in (0, 2):
                        c = 0 if t == 0 else 1
                        tensor.wait_ge(s_la[c], 16)
                        tensor.wait_ge(s_lb[c], 16)
                    lsl = slice(t * 128, (t + 1) * 128)
                    if g >= 2:
                        tensor.wait_ge(s_act, g)
                    pg = ps[g % 2]
                    if g == 0:
                        # split: banks 0-1 then banks 2-3, an inc after
                        # each half so the first exp starts early
                        for half in range(2):
                            for j in range(2 * half, 2 * half + 2):
                                if j % 2 == 0:
                                    tensor.wait_ge(s_ra[j // 2], 16)
                                m0 = j * MCHUNK
                                nc.tensor.matmul(
                                    pg[:, j * MCHUNK : (j + 1) * MCHUNK],
                                    la_sb[:, lsl],
                                    ra_sb[:, m0 : m0 + MCHUNK],
                                    start=True,
                                    stop=False,
                                )
                            for j in range(2 * half, 2 * half + 2):
                                if j % 2 == 0:
                                    tensor.wait_ge(s_rb[j // 2], 16)
                                m0 = j * MCHUNK
                                mm = nc.tensor.matmul(
                                    pg[:, j * MCHUNK : (j + 1) * MCHUNK],
                                    lb_sb[:, lsl],
                                    rb_sb[:, m0 : m0 + MCHUNK],
                                    start=False,
                                    stop=True,
                                )
                                if j % 2 == 1:
                                    mm.then_inc(s_pe, 1)
                        g += 1
                        continue
                    for j in range(CPG):
                        if t == 0 and j % 2 == 0:
                            tensor.wait_ge(s_ra[2 * h + j // 2], 16)
                        m0 = h * ACT_FD + j * MCHUNK
                        nc.tensor.matmul(
                            pg[:, j * MCHUNK : (j + 1) * MCHUNK],
                            la_sb[:, lsl],
                            ra_sb[:, m0 : m0 + MCHUNK],
                            start=True,
                            stop=False,
                        )
                    for j in range(CPG):
                        if t == 0 and j % 2 == 0:
                            tensor.wait_ge(s_rb[2 * h + j // 2], 16)
                        m0 = h * ACT_FD + j * MCHUNK
                        mm = nc.tensor.matmul(
                            pg[:, j * MCHUNK : (j + 1) * MCHUNK],
                            lb_sb[:, lsl],
                            rb_sb[:, m0 : m0 + MCHUNK],
                            start=False,
                            stop=True,
                        )
                        if j == CPG - 1:
                            mm.then_inc(s_pe, 1)
                    g += 1

    return nc


def _bf16_split3(x):
    import ml_dtypes

    bf = ml_dtypes.bfloat16
    x = x.astype(np.float32)
    p1 = x.astype(bf)
    rem = x - p1.astype(np.float32)
    p2 = rem.astype(bf)
    rem2 = rem - p2.astype(np.float32)
    p3 = rem2.astype(bf)
    return p1, p2, p3


def _bandwidth_np(X_fit):
    # mirror of reference._bandwidth (Silverman-style)
    b, n, d = X_fit.shape
    flat = np.asarray(X_fit, dtype=np.float64).reshape(-1)
    q = np.quantile(flat, 0.75) - np.quantile(flat, 0.25)
    std = np.std(np.asarray(X_fit, dtype=np.float64).reshape(b, -1), axis=1, ddof=1)
    return (0.9 * np.minimum(std, q / 1.34) / (n**0.2)).astype(np.float32)


def _host_prep(X_query, X_fit):
    X_query = np.asarray(X_query, dtype=np.float32)
    X_fit = np.asarray(X_fit, dtype=np.float32)
    bw = _bandwidth_np(X_fit)  # [B]

    in_maps = []
    for c in range(NCORES):
        b = c // SHARDS_PER_BATCH
        s = c % SHARDS_PER_BATCH
        XQ = X_query[b, s * NSHARD : (s + 1) * NSHARD]  # [2048, 32]
        XF = X_fit[b]  # [4096, 32]

        # permuted queries: tile t / partition p handles query row p*NT + t
        XQp = XQ.reshape(128, NT, D).transpose(1, 0, 2).reshape(NSHARD, D)
        Q = np.ascontiguousarray((2.0 * XQp.T).astype(np.float32))  # [32, 2048]
        q1, q2, q3 = _bf16_split3(Q)
        FT = np.ascontiguousarray(XF.T.astype(np.float32))  # [32, 4096]
        f1, f2, f3 = _bf16_split3(FT)
        sqr = FT * FT  # f32-rounded squares, matches reference nmu2 terms
        s1, s2, _s3 = _bf16_split3(sqr)

        la_np = np.concatenate([q1, q1, q1], axis=0)  # [96, 2048]
        lb_np = np.concatenate([q2, q2, q3], axis=0)
        ra_np = np.concatenate([f1, f2, f3, s1], axis=0)  # [128, 4096]
        rb_np = np.concatenate([f1, f2, f1, s2], axis=0)

        inv_bw = np.float32(1.0) / bw[b]
        xqn = np.empty((128, NT * D + 1 + NT), dtype=np.float32)
        xqn[:, : NT * D] = XQ.reshape(128, NT * D)
        xqn[:, NT * D] = inv_bw
        xqn[:, NT * D + 1 :] = -inv_bw

        in_maps.append(
            {"la": la_np, "lb": lb_np, "ra": ra_np, "rb": rb_np, "xqn": xqn}
        )
    return in_maps


def _gather(results):
    out = np.empty((B, N), dtype=np.float32)
    for c in range(NCORES):
        b = c // SHARDS_PER_BATCH
        s = c % SHARDS_PER_BATCH
        res = np.asarray(results[c]["res"], dtype=np.float32)  # [128, 16]
        out[b, s * NSHARD : (s + 1) * NSHARD] = res.reshape(NSHARD)
    return out


def kernel(X_query, X_fit):
    from concourse.bass_utils import run_bass_kernel_spmd

    if "nc" not in _cached:
        _cached["nc"] = _build_program()
    nc = _cached["nc"]
    in_maps = _host_prep(X_query, X_fit)
    out = run_bass_kernel_spmd(nc, in_maps, list(range(NCORES)))
    return _gather(out.results)


# revision 66
# speedup vs baseline: 1.0158x; 1.0158x over previous
"""Batched KDE kernel for Trainium2 (8 NeuronCores, SPMD).

Problem: out[b, n] = sum_m exp(-||Xq[b,n] - Xf[b,m]||^2 / bw[b])
  with Silverman bandwidth bw[b] from Xf; b=4, n=m=4096, d=32.

Sharding: data-parallel over batch b (4 batches x 2 shards of query rows
= 8 cores). Each core handles n_shard=2048 query rows against the full
m=4096 fit set of its batch.

Device algorithm (per core), raw Bass with manual semaphores:
  psum[n, m] = 2*dot - nmu2 via TWO bf16 K=128 matmuls per 512-col chunk
  (bf16 streams at 1 col/cycle; f32 values are split into bf16 pieces
  x = x1+x2+x3+O(2^-24); Q = 2*Xq^T, f = Xf^T, s = f32(f^2)):
    mmA: lhsT=[q1; q1; q1; -1]   rhs=[f1; f2; f3; s1]
    mmB: lhsT=[q2; q2; q3; -1]   rhs=[f1; f2; f1; s2]
  Sum = q1(f1+f2+f3) + q2(f1+f2) + q3*f1 - s1 - s2 = Q*f - s up to
  O(2^-17)-level dropped cross terms (~2e-4 relative on the exp args).
  ScalarE activation computes exp(psum/bw - nx2/bw) with a fused
  per-partition accumulate (accum_out) -> the sum over m. ACT is the
  bottleneck engine (~1 elem/lane/cycle @ 1.2 GHz).
  nx2 (query norms) is computed on-device from the raw query rows.
Host does sharding/layout/packing plus the 4 scalar bandwidth values
(the global quantile needs a sort, which is pathological on-device).
"""

import os
import numpy as np

B, N, M, D = 4, 4096, 4096, 32
NCORES = 8
SHARDS_PER_BATCH = NCORES // B  # 2
NSHARD = N // SHARDS_PER_BATCH  # 2048
NT = NSHARD // 128  # 16 n-tiles per core
MCHUNK = 512  # matmul free-dim chunk (one psum bank)
ACT_FD = 2048  # activation free dim (4 psum banks)
NG = NT * (M // ACT_FD)  # 32 matmul/exp groups
CPG = ACT_FD // MCHUNK  # psum banks per group = 4

_cached = {}


def _build_program():
    import concourse.bass as bass
    import concourse.mybir as mybir
    from contextlib import ExitStack

    nc = bass.Bass()
    f32 = mybir.dt.float32
    bf16 = mybir.dt.bfloat16

    # stationary operands: only the 96 data rows come from the host; the
    # -1 rows (96:128) are memset on-device
    la = nc.declare_dram_parameter("la", [96, NSHARD], bf16, isOutput=False)
    lb = nc.declare_dram_parameter("lb", [96, NSHARD], bf16, isOutput=False)
    ra = nc.declare_dram_parameter("ra", [128, M], bf16, isOutput=False)
    rb = nc.declare_dram_parameter("rb", [128, M], bf16, isOutput=False)
    XQN_W = NT * D + 1 + NT
    xqn = nc.declare_dram_parameter("xqn", [128, XQN_W], f32, isOutput=False)
    res = nc.declare_dram_parameter("res", [128, NT], f32, isOutput=True)

    NLC = 2  # 1024-col chunks of la/lb
    NRC = 4  # 1024-col chunks of ra/rb
    LW_ = NSHARD // NLC
    RW = M // NRC

    with ExitStack() as ctx:
        la_sb = ctx.enter_context(nc.sbuf_tensor([128, NSHARD], bf16))
        lb_sb = ctx.enter_context(nc.sbuf_tensor([128, NSHARD], bf16))
        ra_sb = ctx.enter_context(nc.sbuf_tensor([128, M], bf16))
        rb_sb = ctx.enter_context(nc.sbuf_tensor([128, M], bf16))
        xqn_sb = ctx.enter_context(nc.sbuf_tensor([128, XQN_W], f32))
        sq = ctx.enter_context(nc.sbuf_tensor([128, NT * D], f32))
        nx2r = ctx.enter_context(nc.sbuf_tensor([128, NT], f32))
        bias_all = ctx.enter_context(nc.sbuf_tensor([128, NT], f32))
        # slot 0..NG-1 = regular groups; slot NG = the split-off first
        # half-group (banks 0-1 of group 0, summed into res col 0 at the end)
        acc = ctx.enter_context(nc.sbuf_tensor([128, NG + 1], f32))
        res_sb = ctx.enter_context(nc.sbuf_tensor([128, NT], f32))
        warmT = ctx.enter_context(nc.sbuf_tensor([1, 1], f32))
        escr0 = ctx.enter_context(nc.sbuf_tensor([128, ACT_FD], bf16))
        escr1 = ctx.enter_context(nc.sbuf_tensor([128, ACT_FD], bf16))
        escr = [escr0, escr1]
        ps0 = ctx.enter_context(nc.psum_tensor("ps0", [128, ACT_FD], f32))
        ps1 = ctx.enter_context(nc.psum_tensor("ps1", [128, ACT_FD], f32))
        ps = [ps0, ps1]

        sem_xqn = ctx.enter_context(nc.semaphore("sem_xqn"))
        s_la = [ctx.enter_context(nc.semaphore(f"s_la{i}")) for i in range(NLC)]
        s_lb = [ctx.enter_context(nc.semaphore(f"s_lb{i}")) for i in range(NLC)]
        s_ra = [ctx.enter_context(nc.semaphore(f"s_ra{i}")) for i in range(NRC)]
        s_rb = [ctx.enter_context(nc.semaphore(f"s_rb{i}")) for i in range(NRC)]
        sem_out = ctx.enter_context(nc.semaphore("sem_out"))
        s_warm = ctx.enter_context(nc.semaphore("s_warm"))
        s_dve = ctx.enter_context(nc.semaphore("s_dve"))
        s_act = ctx.enter_context(nc.semaphore("s_act"))
        s_pe = ctx.enter_context(nc.semaphore("s_pe"))
        block = ctx.enter_context(nc.Block())

        scale_pos = xqn_sb[:, NT * D : NT * D + 1]  # 1/bw
        neg_invbw = xqn_sb[:, NT * D + 1 : NT * D + 1 + NT]  # -1/bw x NT

        @block.sync
        def _(sync):
            # critical first-half chunks; the rest is deferred until these
            # have landed so they don't compete for HBM bandwidth (the
            # second m-half isn't consumed until halfway through the run)
            sync.dma_start(xqn_sb[:], xqn[:]).then_inc(sem_xqn, 16)
            sync.dma_start(la_sb[0:96, 0:LW_], la[:, 0:LW_]).then_inc(s_la[0], 16)
            sync.dma_start(lb_sb[0:96, 0:LW_], lb[:, 0:LW_]).then_inc(s_lb[0], 16)
            sync.dma_start(ra_sb[:, 0:RW], ra[:, 0:RW]).then_inc(s_ra[0], 16)
            sync.dma_start(rb_sb[:, 0:RW], rb[:, 0:RW]).then_inc(s_rb[0], 16)
            sync.dma_start(
                ra_sb[:, RW : 2 * RW], ra[:, RW : 2 * RW]
            ).then_inc(s_ra[1], 16)
            sync.dma_start(
                rb_sb[:, RW : 2 * RW], rb[:, RW : 2 * RW]
            ).then_inc(s_rb[1], 16)
            sync.wait_ge(s_rb[1], 16)
            sync.dma_start(
                la_sb[0:96, LW_ : 2 * LW_], la[:, LW_ : 2 * LW_]
            ).then_inc(s_la[1], 16)
            sync.dma_start(
                lb_sb[0:96, LW_ : 2 * LW_], lb[:, LW_ : 2 * LW_]
            ).then_inc(s_lb[1], 16)
            for i in range(2, NRC):
                sync.dma_start(
                    ra_sb[:, i * RW : (i + 1) * RW], ra[:, i * RW : (i + 1) * RW]
                ).then_inc(s_ra[i], 16)
                sync.dma_start(
                    rb_sb[:, i * RW : (i + 1) * RW], rb[:, i * RW : (i + 1) * RW]
                ).then_inc(s_rb[i], 16)
            sync.wait_ge(s_dve, 8)
            sync.dma_start(res[:], res_sb[:]).then_inc(sem_out, 16)
            sync.wait_ge(sem_out, 16)

        @block.vector
        def _(vector):
            # scratch for the PE warmup matmuls
            nc.vector.memset(escr0[:, 0:MCHUNK], 0.0).then_inc(s_dve, 1)
            # -1 rows of the stationary operands
            nc.vector.memset(la_sb[96:128, :], -1.0).then_inc(s_dve, 1)
            nc.vector.memset(lb_sb[96:128, :], -1.0).then_inc(s_dve, 1)
            vector.wait_ge(sem_xqn, 16)
            nc.vector.tensor_tensor(
                sq[:],
                xqn_sb[:, : NT * D],
                xqn_sb[:, : NT * D],
                op=mybir.AluOpType.mult,
            ).then_inc(s_dve, 1)
            vector.wait_ge(s_dve, 4)
            nc.vector.tensor_reduce(
                nx2r[:],
                sq[:].rearrange("p (t d) -> p t d", d=D),
                axis=mybir.AxisListType.X,
                op=mybir.AluOpType.add,
            ).then_inc(s_dve, 1)
            vector.wait_ge(s_dve, 5)
            nc.vector.tensor_tensor(
                bias_all[:], nx2r[:], neg_invbw, op=mybir.AluOpType.mult
            ).then_inc(s_dve, 1)
            vector.wait_ge(s_act, NG + 1)
            nc.vector.tensor_reduce(
                res_sb[:],
                acc[:, :NG].rearrange("p (t h) -> p t h", h=M // ACT_FD),
                axis=mybir.AxisListType.X,
                op=mybir.AluOpType.add,
            ).then_inc(s_dve, 1)
            vector.wait_ge(s_dve, 7)
            nc.vector.tensor_tensor(
                res_sb[:, 0:1],
                res_sb[:, 0:1],
                acc[:, NG : NG + 1],
                op=mybir.AluOpType.add,
            ).then_inc(s_dve, 1)

        @block.scalar
        def _(scalar):
            # trigger the exp table-set DMA during the kernel head so the
            # first real exp doesn't pay the ~1.3us ACT_TABLE_LOAD
            nc.scalar.memzero(warmT[:]).then_inc(s_warm, 1)
            scalar.wait_ge(s_warm, 1)
            nc.scalar.activation(
                warmT[:], warmT[:], mybir.ActivationFunctionType.Exp
            )
            scalar.wait_ge(sem_xqn, 16)
            scalar.wait_ge(s_dve, 6)
            # group 0 is split in two so the first exp can start as soon as
            # the first 1024 columns of psum are ready
            scalar.wait_ge(s_pe, 1)
            nc.scalar.activation(
                escr[0][:, 0 : ACT_FD // 2],
                ps[0][:, 0 : ACT_FD // 2],
                mybir.ActivationFunctionType.Exp,
                bias=bias_all[:, 0:1],
                scale=scale_pos,
                accum_out=acc[:, NG : NG + 1],
            ).then_inc(s_act, 1)
            scalar.wait_ge(s_pe, 2)
            nc.scalar.activation(
                escr[0][:, ACT_FD // 2 :],
                ps[0][:, ACT_FD // 2 :],
                mybir.ActivationFunctionType.Exp,
                bias=bias_all[:, 0:1],
                scale=scale_pos,
                accum_out=acc[:, 0:1],
            ).then_inc(s_act, 1)
            for g in range(1, NG):
                t = g % NT
                scalar.wait_ge(s_pe, g + 2)
                slot = t * (M // ACT_FD) + (g // NT)
                nc.scalar.activation(
                    escr[g % 2][:],
                    ps[g % 2][:],
                    mybir.ActivationFunctionType.Exp,
                    bias=bias_all[:, t : t + 1],
                    scale=scale_pos,
                    accum_out=acc[:, slot : slot + 1],
                ).then_inc(s_act, 1)

        @block.tensor
        def _(tensor):
            # warm the PE clock (HAM) during the head with dummy matmuls on
            # the memset -1 rows, so the first real groups run at 2.4 GHz
            tensor.wait_ge(s_dve, 1)  # warmup scratch memset
            for _w in range(16):
                nc.tensor.matmul(
                    ps0[:, 0:MCHUNK],
                    escr0[:, 0:128],
                    escr0[:, 0:MCHUNK],
                    start=True,
                    stop=True,
                )
            tensor.wait_ge(s_dve, 3)  # la/lb -1 rows memset
            g = 0
            # h-outer: the second m-half (ra/rb chunks 2-3) is first
            # touched halfway through the kernel, so its DMA is deferred
            for h in range(M // ACT_FD):
                for t in range(NT):
                    if t % (NT // NLC) == 0:
                        c = t // (NT // NLC)
                        tensor.wait_ge(s_la[c], 16)
                        tensor.wait_ge(s_lb[c], 16)
                    lsl = slice(t * 128, (t + 1) * 128)
                    if g >= 2:
                        tensor.wait_ge(s_act, g)
                    pg = ps[g % 2]
                    if g == 0:
                        # split: banks 0-1 then banks 2-3, an inc after
                        # each half so the first exp starts early
                        for half in range(2):
                            for j in range(2 * half, 2 * half + 2):
                                if j % 2 == 0:
                                    tensor.wait_ge(s_ra[j // 2], 16)
                                m0 = j * MCHUNK
                                nc.tensor.matmul(
                                    pg[:, j * MCHUNK : (j + 1) * MCHUNK],
                                    la_sb[:, lsl],
                                    ra_sb[:, m0 : m0 + MCHUNK],
                                    start=True,
                                    stop=False,
                                )
                            for j in range(2 * half, 2 * half + 2):
                                if j % 2 == 0:
                                    tensor.wait_ge(s_rb[j // 2], 16)
                                m0 = j * MCHUNK
                                mm = nc.tensor.matmul(
                                    pg[:, j * MCHUNK : (j + 1) * MCHUNK],
                                    lb_sb[:, lsl],
                                    rb_sb[:, m0 : m0 + MCHUNK],
                                    start=False,
                                    stop=True,
                                )
                                if j % 2 == 1:
                                    mm.then_inc(s_pe, 1)
                        g += 1
                        continue
                    for j in range(CPG):
                        if t == 0 and j % 2 == 0:
                            tensor.wait_ge(s_ra[2 * h + j // 2], 16)
                        m0 = h * ACT_FD + j * MCHUNK
                        nc.tensor.matmul(
                            pg[:, j * MCHUNK : (j + 1) * MCHUNK],
                            la_sb[:, lsl],
                            ra_sb[:, m0 : m0 + MCHUNK],
                            start=True,
                            stop=False,
                        )
                    for j in range(CPG):
                        if t == 0 and j % 2 == 0:
                            tensor.wait_ge(s_rb[2 * h + j // 2], 16)
                        m0 = h * ACT_FD + j * MCHUNK
                        mm = nc.tensor.matmul(
                            pg[:, j * MCHUNK : (j + 1) * MCHUNK],
                            lb_sb[:, lsl],
                            rb_sb[:, m0 : m0 + MCHUNK],
                            start=False,
                            stop=True,
                        )
                        if j == CPG - 1:
                            mm.then_inc(s_pe, 1)
                    g += 1

    return nc


def _bf16_split3(x):
    import ml_dtypes

    bf = ml_dtypes.bfloat16
    x = x.astype(np.float32)
    p1 = x.astype(bf)
    rem = x - p1.astype(np.float32)
    p2 = rem.astype(bf)
    rem2 = rem - p2.astype(np.float32)
    p3 = rem2.astype(bf)
    return p1, p2, p3


def _bandwidth_np(X_fit):
    # mirror of reference._bandwidth (Silverman-style)
    b, n, d = X_fit.shape
    flat = np.asarray(X_fit, dtype=np.float64).reshape(-1)
    q = np.quantile(flat, 0.75) - np.quantile(flat, 0.25)
    std = np.std(np.asarray(X_fit, dtype=np.float64).reshape(b, -1), axis=1, ddof=1)
    return (0.9 * np.minimum(std, q / 1.34) / (n**0.2)).astype(np.float32)


def _host_prep(X_query, X_fit):
    X_query = np.asarray(X_query, dtype=np.float32)
    X_fit = np.asarray(X_fit, dtype=np.float32)
    bw = _bandwidth_np(X_fit)  # [B]

    in_maps = []
    for c in range(NCORES):
        b = c // SHARDS_PER_BATCH
        s = c % SHARDS_PER_BATCH
        XQ = X_query[b, s * NSHARD : (s + 1) * NSHARD]  # [2048, 32]
        XF = X_fit[b]  # [4096, 32]

        # permuted queries: tile t / partition p handles query row p*NT + t
        XQp = XQ.reshape(128, NT, D).transpose(1, 0, 2).reshape(NSHARD, D)
        Q = np.ascontiguousarray((2.0 * XQp.T).astype(np.float32))  # [32, 2048]
        q1, q2, q3 = _bf16_split3(Q)
        FT = np.ascontiguousarray(XF.T.astype(np.float32))  # [32, 4096]
        f1, f2, f3 = _bf16_split3(FT)
        sqr = FT * FT  # f32-rounded squares, matches reference nmu2 terms
        s1, s2, _s3 = _bf16_split3(sqr)

        la_np = np.concatenate([q1, q1, q1], axis=0)  # [96, 2048]
        lb_np = np.concatenate([q2, q2, q3], axis=0)
        ra_np = np.concatenate([f1, f2, f3, s1], axis=0)  # [128, 4096]
        rb_np = np.concatenate([f1, f2, f1, s2], axis=0)

        inv_bw = np.float32(1.0) / bw[b]
        xqn = np.empty((128, NT * D + 1 + NT), dtype=np.float32)
        xqn[:, : NT * D] = XQ.reshape(128, NT * D)
        xqn[:, NT * D] = inv_bw
        xqn[:, NT * D + 1 :] = -inv_bw

        in_maps.append(
            {"la": la_np, "lb": lb_np, "ra": ra_np, "rb": rb_np, "xqn": xqn}
        )
    return in_maps


def _gather(results):
    out = np.empty((B, N), dtype=np.float32)
    for c in range(NCORES):
        b = c // SHARDS_PER_BATCH
        s = c % SHARDS_PER_BATCH
        res = np.asarray(results[c]["res"], dtype=np.float32)  # [128, 16]
        out[b, s * NSHARD : (s + 1) * NSHARD] = res.reshape(NSHARD)
    return out


def kernel(X_query, X_fit):
    from concourse.bass_utils import run_bass_kernel_spmd

    if "nc" not in _cached:
        _cached["nc"] = _build_program()
    nc = _cached["nc"]
    in_maps = _host_prep(X_query, X_fit)
    out = run_bass_kernel_spmd(nc, in_maps, list(range(NCORES)))
    return _gather(out.results)
